# revision 20
# baseline (speedup 1.0000x reference)
"""AttentiveMemory sparse-attention kernel for 8 TRN2 NeuronCores.

Data-parallel over rows of x (2048 rows/core); memMatrix and the projection
weights are replicated.  Per core:

    query^T = Wq^T.T @ x^T          (split-bf16 x3 matmuls, fp32 accumulate)
    key^T   = Wk^T.T @ mem^T        (split-bf16 x3; sharded over cores by
                                     memory slot + AllGather)
    logits  = query^T.T @ key^T     (split-bf16 x3) -> PSUM fp32
    z       = exp(logits/32)        (ACT, straight from PSUM; logits are tiny
                                     so no max-subtraction is needed)
    t5      = 5th largest z         (vector.max top-8, one instruction)
    u       = z * (z > t5)          (fused scalar_tensor_tensor, sum as accum)
    att     = u / (sum(u) + eps)    -> DRAM
    out     = (u @ mem) / (sum(u) + eps)  (bf16 matmul on u^T) -> DRAM

The split-bf16 trick (a@b ~ ah@bh + ah@bl + al@bh with a = ah + al) gives
fp32-grade logits at 3 bf16-rate matmuls; plain bf16/fp16/tf32 logits flip
the top-5 selection on too many rows to pass the rel-err gate.

key^T depends only on replicated inputs, so each core computes the 256
slots it owns (by partition id) and an AllGather through DRAM bounce
buffers distributes the full key^T while the x-side transposes and the
query projection keep the engines busy.
"""
from contextlib import ExitStack

import numpy as np

import concourse.bass as bass
import concourse.mybir as mybir
import concourse.tile as tile
from concourse import bacc
from concourse.masks import make_identity
from concourse.bass_utils import run_bass_kernel_spmd

N_CORES = 8
N_FULL = 16384
R = N_FULL // N_CORES   # 2048 rows per core
C = 1024                # feature dim
Q = 512                 # qk dim
S = 2048                # memory slots
SCALE = 1.0 / 32.0      # 1/sqrt(C)
EPS = 1e-12
S_OWN = S // N_CORES    # 256 slots of key^T computed per core

FP32 = mybir.dt.float32
BF16 = mybir.dt.bfloat16
AF = mybir.ActivationFunctionType
OP = mybir.AluOpType

RT = R // 128   # 16
CT = C // 128   # 8
QT = Q // 128   # 4
ST = S // 128   # 16

SHARD_KT = True

ident_g = [None]


def _load_split_transpose(nc, tc, stack, ext, row0, n_rows_tiles, T_hi, T_lo,
                          free_base, prefix, preloaded=None, ld_bufs=5,
                          sp_bufs=2):
    """Load `n_rows_tiles` (128, C) fp32 tiles of `ext` starting at row tile
    `row0`, split into bf16 hi/lo, PE-transpose into T_hi/T_lo[ci] at free
    offset `free_base`."""
    GROUP = 4
    lpool = stack.enter_context(tc.tile_pool(name=f"{prefix}_ld", bufs=ld_bufs))
    spool = stack.enter_context(tc.tile_pool(name=f"{prefix}_sp", bufs=sp_bufs))
    pspool = stack.enter_context(
        tc.tile_pool(name=f"{prefix}_ps", bufs=2, space="PSUM"))
    n_groups = n_rows_tiles // GROUP
    for g in range(n_groups):
        his, los = [], []
        for i in range(GROUP):
            rt = row0 + g * GROUP + i
            if preloaded is not None and g * GROUP + i < len(preloaded):
                t = preloaded[g * GROUP + i]
            else:
                t = lpool.tile([128, C], FP32, tag="ld", name="ld")
                nc.sync.dma_start(t[:], ext[rt * 128:(rt + 1) * 128, :])
            hi = spool.tile([128, C], BF16, tag=f"h{i}", name=f"h{i}")
            lo = spool.tile([128, C], BF16, tag=f"l{i}", name=f"l{i}")
            nc.scalar.copy(hi[:], t[:])
            nc.vector.tensor_tensor(lo[:], t[:], hi[:], OP.subtract)
            his.append(hi)
            los.append(lo)
        fb = free_base + g * GROUP * 128
        for ci in range(len(T_hi)):
            for tiles, T in ((his, T_hi), (los, T_lo)):
                ps = pspool.tile([128, GROUP * 128], BF16, tag="ps", name="ps")
                for i in range(GROUP):
                    nc.tensor.transpose(
                        ps[:, i * 128:(i + 1) * 128],
                        tiles[i][:, ci * 128:(ci + 1) * 128], ident_g[0][:])
                nc.vector.tensor_copy(T[ci][:, fb:fb + GROUP * 128], ps[:])


def _project(nc, tc, stack, wT_hi, wT_lo, inT_hi, inT_lo, oT_hi, oT_lo,
             n_free, free_base, prefix, chunk=512, rhs_hi=None, rhs_lo=None):
    """o^T = W^T.T @ in^T with split-bf16 x3; writes oT_*[mi][:, free_base:...]."""
    pspool = stack.enter_context(
        tc.tile_pool(name=f"{prefix}_qps", bufs=4, space="PSUM"))
    if rhs_hi is None:
        rhs_hi = lambda ci, ni, ch: inT_hi[ci][:, ni * ch:(ni + 1) * ch]
        rhs_lo = lambda ci, ni, ch: inT_lo[ci][:, ni * ch:(ni + 1) * ch]
    ct = len(wT_hi)
    for ni in range(n_free // chunk):
        for mi in range(len(oT_hi)):
            ps = pspool.tile([128, chunk], FP32, tag="ps", name="ps")
            n_mm = ct * 3
            k = 0
            for ci in range(ct):
                for (wa, xf) in ((wT_hi, rhs_hi), (wT_hi, rhs_lo),
                                 (wT_lo, rhs_hi)):
                    nc.tensor.matmul(
                        ps[:],
                        wa[ci][:, mi * 128:(mi + 1) * 128],
                        xf(ci, ni, chunk),
                        start=(k == 0), stop=(k == n_mm - 1))
                    k += 1
            dst = slice(free_base + ni * chunk, free_base + (ni + 1) * chunk)
            nc.scalar.copy(oT_hi[mi][:, dst], ps[:])
            nc.vector.tensor_tensor(
                oT_lo[mi][:, dst], ps[:], oT_hi[mi][:, dst], OP.subtract)


def build():
    nc = bacc.Bacc()
    x_ext = nc.declare_dram_parameter("x", [R, C], FP32, isOutput=False)
    wq_ext = nc.declare_dram_parameter("W_q", [Q, C], FP32, isOutput=False)
    wk_ext = nc.declare_dram_parameter("W_k", [Q, C], FP32, isOutput=False)
    mem_ext = nc.declare_dram_parameter("memMatrix", [S, C], FP32, isOutput=False)
    out_ext = nc.declare_dram_parameter("out", [R, C], FP32, isOutput=True)
    att_ext = nc.declare_dram_parameter("att", [R, S], FP32, isOutput=True)

    with tile.TileContext(nc) as tc, ExitStack() as top:
        const_pool = top.enter_context(tc.tile_pool(name="const", bufs=1))
        ident = const_pool.tile([128, 128], BF16)
        make_identity(nc, ident[:])
        ident_g[0] = ident

        qk_pool = top.enter_context(tc.tile_pool(name="qk", bufs=1))
        memhi_pool = top.enter_context(tc.tile_pool(name="memhi", bufs=1))
        qT_hi = [[qk_pool.tile([128, R // 2], BF16, tag=f"qTh{h}_{m}",
                               name=f"qTh{h}_{m}") for m in range(QT)]
                 for h in range(2)]
        qT_lo = [[qk_pool.tile([128, R // 2], BF16, tag=f"qTl{h}_{m}",
                               name=f"qTl{h}_{m}") for m in range(QT)]
                 for h in range(2)]
        kT_hi = [qk_pool.tile([128, S], BF16, tag=f"kTh{m}", name=f"kTh{m}")
                 for m in range(QT)]
        kT_lo = [qk_pool.tile([128, S], BF16, tag=f"kTl{m}", name=f"kTl{m}")
                 for m in range(QT)]
        mem_hi = [memhi_pool.tile([128, C], BF16, tag=f"mh{s}", name=f"mh{s}")
                  for s in range(ST)]

        with ExitStack() as wq_stack:
            wq_pool = wq_stack.enter_context(tc.tile_pool(name="wqT", bufs=1))
            wqT_hi = [wq_pool.tile([128, Q], BF16, tag=f"h{c}", name=f"h{c}")
                      for c in range(CT)]
            wqT_lo = [wq_pool.tile([128, Q], BF16, tag=f"l{c}", name=f"l{c}")
                      for c in range(CT)]

            # ---- KEY^T first so the AllGather is in flight while the
            # x-side work keeps the engines busy ----
            with ExitStack() as wk_stack:
                wk_pool = wk_stack.enter_context(tc.tile_pool(name="wkT", bufs=1))
                wkT_hi = [wk_pool.tile([128, Q], BF16, tag=f"h{c}", name=f"h{c}")
                          for c in range(CT)]
                wkT_lo = [wk_pool.tile([128, Q], BF16, tag=f"l{c}", name=f"l{c}")
                          for c in range(CT)]
                with ExitStack() as st:
                    _load_split_transpose(nc, tc, st, wk_ext, 0, QT,
                                          wkT_hi, wkT_lo, 0, "wk")

                if SHARD_KT:
                    # compute key^T only for the S_OWN slots this core owns,
                    # AllGather the rest while X/QUERY run.
                    ccpool = top.enter_context(
                        tc.tile_pool(name="ccdram", bufs=1, space="DRAM"))
                    kin = ccpool.tile([2 * Q, S_OWN], BF16, name="kin")
                    kout = ccpool.tile([N_CORES, 2 * Q, S_OWN], BF16,
                                       name="kout", addr_space="Shared")
                    pid = nc.sync.partition_id()
                    with ExitStack() as st:
                        spool = st.enter_context(
                            tc.tile_pool(name="kslc", bufs=1))
                        pspool = st.enter_context(
                            tc.tile_pool(name="kslc_ps", bufs=2, space="PSUM"))
                        mTs_hi = [spool.tile([128, S_OWN], BF16, tag=f"th{c}",
                                             name=f"th{c}") for c in range(CT)]
                        mTs_lo = [spool.tile([128, S_OWN], BF16, tag=f"tl{c}",
                                             name=f"tl{c}") for c in range(CT)]
                        ohis, olos = [], []
                        for k2 in range(S_OWN // 128):
                            t = spool.tile([128, C], FP32, tag=f"od{k2}",
                                           name=f"od{k2}")
                            nc.sync.dma_start(
                                t[:],
                                mem_ext[bass.ds(pid * S_OWN + k2 * 128, 128), :])
                            ohi = spool.tile([128, C], BF16, tag=f"oh{k2}",
                                             name=f"oh{k2}")
                            olo = spool.tile([128, C], BF16, tag=f"ol{k2}",
                                             name=f"ol{k2}")
                            nc.scalar.copy(ohi[:], t[:])
                            nc.vector.tensor_tensor(olo[:], t[:], ohi[:],
                                                    OP.subtract)
                            ohis.append(ohi)
                            olos.append(olo)
                        for ci in range(CT):
                            for tiles, T in ((ohis, mTs_hi), (olos, mTs_lo)):
                                ps = pspool.tile([128, S_OWN], BF16, tag="ps",
                                                 name="ps")
                                for i in range(S_OWN // 128):
                                    nc.tensor.transpose(
                                        ps[:, i * 128:(i + 1) * 128],
                                        tiles[i][:, ci * 128:(ci + 1) * 128],
                                        ident[:])
                                nc.vector.tensor_copy(T[ci][:], ps[:])
                        # key^T slice + ship to bounce buffer
                        kps = st.enter_context(
                            tc.tile_pool(name="kps", bufs=2, space="PSUM"))
                        for mi in range(QT):
                            ps = kps.tile([128, S_OWN], FP32, tag="ps", name="ps")
                            n_mm = CT * 3
                            k = 0
                            for ci in range(CT):
                                for (wa, xa) in ((wkT_hi, mTs_hi),
                                                 (wkT_hi, mTs_lo),
                                                 (wkT_lo, mTs_hi)):
                                    nc.tensor.matmul(
                                        ps[:],
                                        wa[ci][:, mi * 128:(mi + 1) * 128],
                                        xa[ci][:],
                                        start=(k == 0), stop=(k == n_mm - 1))
                                    k += 1
                            sh = spool.tile([128, S_OWN], BF16, tag=f"sh{mi}",
                                            name=f"sh{mi}")
                            sl = spool.tile([128, S_OWN], BF16, tag=f"sl{mi}",
                                            name=f"sl{mi}")
                            nc.scalar.copy(sh[:], ps[:])
                            nc.vector.tensor_tensor(sl[:], ps[:], sh[:],
                                                    OP.subtract)
                            nc.sync.dma_start(
                                kin[mi * 128:(mi + 1) * 128, :], sh[:])
                            nc.sync.dma_start(
                                kin[(QT + mi) * 128:(QT + mi + 1) * 128, :], sl[:])
                        nc.gpsimd.collective_compute(
                            "AllGather", OP.bypass,
                            replica_groups=[list(range(N_CORES))],
                            ins=[kin[:]], outs=[kout[:]])
                else:
                    with ExitStack() as mst:
                        mT_pool = mst.enter_context(
                            tc.tile_pool(name="mT", bufs=1))
                        mT_hi = [mT_pool.tile([128, S // 2], BF16, tag=f"h{c}",
                                              name=f"h{c}") for c in range(CT)]
                        mT_lo = [mT_pool.tile([128, S // 2], BF16, tag=f"l{c}",
                                              name=f"l{c}") for c in range(CT)]
                        for half in range(2):
                            with ExitStack() as st:
                                _load_split_transpose(
                                    nc, tc, st, mem_ext, half * (ST // 2),
                                    ST // 2, mT_hi, mT_lo, 0, f"m{half}")
                            with ExitStack() as st:
                                _project(nc, tc, st, wkT_hi, wkT_lo,
                                         mT_hi, mT_lo, kT_hi, kT_lo,
                                         S // 2, half * (S // 2), f"k{half}")

            with ExitStack() as st:
                _load_split_transpose(nc, tc, st, wq_ext, 0, QT,
                                      wqT_hi, wqT_lo, 0, "wq")

            # ---- X + QUERY h0, then phase B rows 0..7 while the other
            # half of x is still being projected ----
            with ExitStack() as xst:
                xT_pool = xst.enter_context(tc.tile_pool(name="xT", bufs=1))
                # row-tile-major transposed layout: [c_local, rt, ci, r_local]
                # so one DMA xbar transpose per x row tile writes a contiguous
                # (128, CT, 128) block (PE + DVE stay off this path)
                xTf_hi = xT_pool.tile([128, (RT // 2) * C], BF16, tag="fh",
                                      name="fh")
                xTf_lo = xT_pool.tile([128, (RT // 2) * C], BF16, tag="fl",
                                      name="fl")

                def x_rhs(T):
                    t4 = T[:].rearrange("p (rt ci r) -> p rt ci r",
                                        rt=RT // 2, ci=CT)
                    def fn(ci, ni, chunk):
                        g = (ni * chunk) // 128
                        ng = chunk // 128
                        return t4[:, g:g + ng, ci, :]
                    return fn

                def do_x_half(half):
                    with ExitStack() as st:
                        lpool = st.enter_context(
                            tc.tile_pool(name=f"x{half}_ld", bufs=5))
                        spool = st.enter_context(
                            tc.tile_pool(name=f"x{half}_sp", bufs=3))
                        for i in range(RT // 2):
                            rt = half * (RT // 2) + i
                            t = lpool.tile([128, C], FP32, tag="ld", name="ld")
                            nc.sync.dma_start(
                                t[:], x_ext[rt * 128:(rt + 1) * 128, :])
                            hi = spool.tile([128, C], BF16, tag="h", name="h")
                            lo = spool.tile([128, C], BF16, tag="l", name="l")
                            nc.scalar.copy(hi[:], t[:])
                            nc.vector.tensor_tensor(lo[:], t[:], hi[:],
                                                    OP.subtract)
                            dst = slice(i * C, (i + 1) * C)
                            nc.sync.dma_start_transpose(
                                xTf_hi[:, dst].rearrange("p (b r) -> p b r",
                                                         b=CT), hi[:])
                            nc.sync.dma_start_transpose(
                                xTf_lo[:, dst].rearrange("p (b r) -> p b r",
                                                         b=CT), lo[:])
                    with ExitStack() as st:
                        _project(nc, tc, st, wqT_hi, wqT_lo, None, None,
                                 qT_hi[half], qT_lo[half], R // 2, 0,
                                 f"q{half}", rhs_hi=x_rhs(xTf_hi),
                                 rhs_lo=x_rhs(xTf_lo))

                do_x_half(0)

                # ---- MEM native hi (all slots; rhs of the out matmul) ----
                with tc.tile_pool(name="m_ld", bufs=3) as mload:
                    for si in range(ST):
                        t = mload.tile([128, C], FP32, tag="ld", name="ld")
                        nc.sync.dma_start(t[:],
                                          mem_ext[si * 128:(si + 1) * 128, :])
                        nc.scalar.copy(mem_hi[si][:], t[:])

                if SHARD_KT:
                    # pull the gathered key^T out of the bounce buffer
                    for split, dsts in ((0, kT_hi), (1, kT_lo)):
                        for mi in range(QT):
                            row = (split * QT + mi) * 128
                            srcap = kout[:, row:row + 128, :].rearrange(
                                "g q s -> q g s")
                            nc.sync.dma_start(
                                dsts[mi][:].rearrange("q (g s) -> q g s",
                                                      g=N_CORES), srcap)

                def b_block(ri_list):
                    with tc.tile_pool(name="batt", bufs=2) as bpool, \
                         tc.tile_pool(name="ps_att", bufs=1,
                                      space="PSUM") as ps_att_pool, \
                         tc.tile_pool(name="ps_ut", bufs=1,
                                      space="PSUM") as ps_ut_pool, \
                         tc.tile_pool(name="ps_out", bufs=1,
                                      space="PSUM") as ps_out_pool:
                        pend = None

                        def emit_post(st):
                            u, recip, ri = st["u"], st["recip"], st["ri"]
                            ps_ut = ps_ut_pool.tile([128, S], BF16, tag="ut",
                                                    name="ut")
                            for sj in range(ST):
                                nc.tensor.transpose(
                                    ps_ut[:, sj * 128:(sj + 1) * 128],
                                    u[:, sj * 128:(sj + 1) * 128], ident[:])
                            uT = bpool.tile([128, S], BF16, tag="uT", name="uT")
                            nc.vector.tensor_copy(uT[:], ps_ut[:])
                            ps_o = ps_out_pool.tile([128, C], FP32, tag="o",
                                                    name="o")
                            for nj in range(C // 512):
                                for sj in range(ST):
                                    nc.tensor.matmul(
                                        ps_o[:, nj * 512:(nj + 1) * 512],
                                        uT[:, sj * 128:(sj + 1) * 128],
                                        mem_hi[sj][:, nj * 512:(nj + 1) * 512],
                                        start=(sj == 0), stop=(sj == ST - 1))
                            out_sb = bpool.tile([128, C], FP32, tag="outsb",
                                                name="outsb")
                            nc.scalar.mul(out_sb[:], ps_o[:], recip[:])
                            nc.sync.dma_start(
                                out_ext[ri * 128:(ri + 1) * 128, :], out_sb[:])

                        for ri in ri_list:
                            ps_att = ps_att_pool.tile([128, S], FP32, tag="att",
                                                      name="att")
                            z = bpool.tile([128, S], FP32, tag="z", name="z")
                            rh, rc = ri // (RT // 2), ri % (RT // 2)
                            for sc in range(S // 512):
                                k = 0
                                n_mm = QT * 3
                                for mi in range(QT):
                                    for (qa, ka) in ((qT_hi[rh], kT_hi),
                                                     (qT_hi[rh], kT_lo),
                                                     (qT_lo[rh], kT_hi)):
                                        nc.tensor.matmul(
                                            ps_att[:, sc * 512:(sc + 1) * 512],
                                            qa[mi][:, rc * 128:(rc + 1) * 128],
                                            ka[mi][:, sc * 512:(sc + 1) * 512],
                                            start=(k == 0), stop=(k == n_mm - 1))
                                        k += 1
                                nc.scalar.activation(
                                    z[:, sc * 512:(sc + 1) * 512],
                                    ps_att[:, sc * 512:(sc + 1) * 512],
                                    AF.Exp, bias=0.0, scale=float(SCALE))

                            if pend is not None:
                                emit_post(pend)

                            t8 = bpool.tile([128, 8], FP32, tag="t8", name="t8")
                            nc.vector.max(t8[:], z[:])
                            u = bpool.tile([128, S], BF16, tag="u", name="u")
                            ssum = bpool.tile([128, 1], FP32, tag="ssum",
                                              name="ssum")
                            nc.vector.scalar_tensor_tensor(
                                out=u[:], in0=z[:], scalar=t8[:, 4:5], in1=z[:],
                                op0=OP.is_gt, op1=OP.mult, accum_out=ssum[:])
                            rin = bpool.tile([128, 1], FP32, tag="rin",
                                             name="rin")
                            nc.vector.tensor_scalar_add(rin[:], ssum[:],
                                                        float(EPS))
                            recip = bpool.tile([128, 1], FP32, tag="recip",
                                               name="recip")
                            nc.vector.reciprocal(recip[:], rin[:])
                            att_sb = bpool.tile([128, S], FP32, tag="attsb",
                                                name="attsb")
                            nc.scalar.mul(att_sb[:], u[:], recip[:])
                            nc.sync.dma_start(
                                att_ext[ri * 128:(ri + 1) * 128, :], att_sb[:])
                            pend = {"u": u, "recip": recip, "ri": ri}
                        emit_post(pend)

                b_block(range(0, RT // 2))
                do_x_half(1)
                b_block(range(RT // 2, RT))

    nc.finalize()
    return nc


_NC_CACHE = None


def kernel(x, W_q, W_k, memMatrix):
    global _NC_CACHE
    if _NC_CACHE is None:
        _NC_CACHE = build()
    nc = _NC_CACHE
    x = np.ascontiguousarray(np.asarray(x, dtype=np.float32))
    W_q = np.ascontiguousarray(np.asarray(W_q, dtype=np.float32))
    W_k = np.ascontiguousarray(np.asarray(W_k, dtype=np.float32))
    memMatrix = np.ascontiguousarray(np.asarray(memMatrix, dtype=np.float32))
    in_maps = [
        {"x": x[i * R:(i + 1) * R], "W_q": W_q, "W_k": W_k, "memMatrix": memMatrix}
        for i in range(N_CORES)
    ]
    res = run_bass_kernel_spmd(nc, in_maps, core_ids=list(range(N_CORES)))
    out = np.concatenate([res.results[i]["out"] for i in range(N_CORES)], axis=0)
    att = np.concatenate([res.results[i]["att"] for i in range(N_CORES)], axis=0)
    return out, att


# revision 21
# speedup vs baseline: 1.0578x; 1.0578x over previous
"""AttentiveMemory sparse-attention kernel for 8 TRN2 NeuronCores.

Data-parallel over rows of x (2048 rows/core); memMatrix and the projection
weights are replicated.  Per core:

    query^T = Wq^T.T @ x^T          (split-bf16 x3 matmuls, fp32 accumulate)
    key^T   = Wk^T.T @ mem^T        (split-bf16 x3; sharded over cores by
                                     memory slot + AllGather)
    logits  = query^T.T @ key^T     (split-bf16 x3) -> PSUM fp32
    z       = exp(logits/32)        (ACT, straight from PSUM; logits are tiny
                                     so no max-subtraction is needed)
    t5      = 5th largest z         (vector.max top-8, one instruction)
    u       = z * (z > t5)          (fused scalar_tensor_tensor, sum as accum)
    att     = u / (sum(u) + eps)    -> DRAM
    out     = (u @ mem) / (sum(u) + eps)  (bf16 matmul on u^T) -> DRAM

The split-bf16 trick (a@b ~ ah@bh + ah@bl + al@bh with a = ah + al) gives
fp32-grade logits at 3 bf16-rate matmuls; plain bf16/fp16/tf32 logits flip
the top-5 selection on too many rows to pass the rel-err gate.

key^T depends only on replicated inputs, so each core computes the 256
slots it owns (by partition id) and an AllGather through DRAM bounce
buffers distributes the full key^T while the x-side transposes and the
query projection keep the engines busy.
"""
from contextlib import ExitStack

import numpy as np

import concourse.bass as bass
import concourse.mybir as mybir
import concourse.tile as tile
from concourse import bacc
from concourse.masks import make_identity
from concourse.bass_utils import run_bass_kernel_spmd

N_CORES = 8
N_FULL = 16384
R = N_FULL // N_CORES   # 2048 rows per core
C = 1024                # feature dim
Q = 512                 # qk dim
S = 2048                # memory slots
SCALE = 1.0 / 32.0      # 1/sqrt(C)
EPS = 1e-12
S_OWN = S // N_CORES    # 256 slots of key^T computed per core

FP32 = mybir.dt.float32
BF16 = mybir.dt.bfloat16
AF = mybir.ActivationFunctionType
OP = mybir.AluOpType

RT = R // 128   # 16
CT = C // 128   # 8
QT = Q // 128   # 4
ST = S // 128   # 16

SHARD_KT = True

ident_g = [None]


def _load_split_transpose(nc, tc, stack, ext, row0, n_rows_tiles, T_hi, T_lo,
                          free_base, prefix, preloaded=None, ld_bufs=5,
                          sp_bufs=2):
    """Load `n_rows_tiles` (128, C) fp32 tiles of `ext` starting at row tile
    `row0`, split into bf16 hi/lo, PE-transpose into T_hi/T_lo[ci] at free
    offset `free_base`."""
    GROUP = 4
    lpool = stack.enter_context(tc.tile_pool(name=f"{prefix}_ld", bufs=ld_bufs))
    spool = stack.enter_context(tc.tile_pool(name=f"{prefix}_sp", bufs=sp_bufs))
    pspool = stack.enter_context(
        tc.tile_pool(name=f"{prefix}_ps", bufs=2, space="PSUM"))
    n_groups = n_rows_tiles // GROUP
    for g in range(n_groups):
        his, los = [], []
        for i in range(GROUP):
            rt = row0 + g * GROUP + i
            if preloaded is not None and g * GROUP + i < len(preloaded):
                t = preloaded[g * GROUP + i]
            else:
                t = lpool.tile([128, C], FP32, tag="ld", name="ld")
                nc.sync.dma_start(t[:], ext[rt * 128:(rt + 1) * 128, :])
            hi = spool.tile([128, C], BF16, tag=f"h{i}", name=f"h{i}")
            lo = spool.tile([128, C], BF16, tag=f"l{i}", name=f"l{i}")
            nc.scalar.copy(hi[:], t[:])
            nc.vector.tensor_tensor(lo[:], t[:], hi[:], OP.subtract)
            his.append(hi)
            los.append(lo)
        fb = free_base + g * GROUP * 128
        for ci in range(len(T_hi)):
            for tiles, T in ((his, T_hi), (los, T_lo)):
                ps = pspool.tile([128, GROUP * 128], BF16, tag="ps", name="ps")
                for i in range(GROUP):
                    nc.tensor.transpose(
                        ps[:, i * 128:(i + 1) * 128],
                        tiles[i][:, ci * 128:(ci + 1) * 128], ident_g[0][:])
                nc.vector.tensor_copy(T[ci][:, fb:fb + GROUP * 128], ps[:])


def _project(nc, tc, stack, wT_hi, wT_lo, inT_hi, inT_lo, oT_hi, oT_lo,
             n_free, free_base, prefix, chunk=512):
    """o^T = W^T.T @ in^T with split-bf16 x3; writes oT_*[mi][:, free_base:...]."""
    pspool = stack.enter_context(
        tc.tile_pool(name=f"{prefix}_qps", bufs=4, space="PSUM"))
    ct = len(wT_hi)
    for mi in range(len(oT_hi)):
        for ni in range(n_free // chunk):
            ps = pspool.tile([128, chunk], FP32, tag="ps", name="ps")
            n_mm = ct * 3
            k = 0
            for ci in range(ct):
                for (wa, xa) in ((wT_hi, inT_hi), (wT_hi, inT_lo),
                                 (wT_lo, inT_hi)):
                    nc.tensor.matmul(
                        ps[:],
                        wa[ci][:, mi * 128:(mi + 1) * 128],
                        xa[ci][:, ni * chunk:(ni + 1) * chunk],
                        start=(k == 0), stop=(k == n_mm - 1))
                    k += 1
            dst = slice(free_base + ni * chunk, free_base + (ni + 1) * chunk)
            nc.scalar.copy(oT_hi[mi][:, dst], ps[:])
            nc.vector.tensor_tensor(
                oT_lo[mi][:, dst], ps[:], oT_hi[mi][:, dst], OP.subtract)


def build():
    nc = bacc.Bacc()
    x_ext = nc.declare_dram_parameter("x", [R, C], FP32, isOutput=False)
    wq_ext = nc.declare_dram_parameter("W_q", [Q, C], FP32, isOutput=False)
    wk_ext = nc.declare_dram_parameter("W_k", [Q, C], FP32, isOutput=False)
    mem_ext = nc.declare_dram_parameter("memMatrix", [S, C], FP32, isOutput=False)
    out_ext = nc.declare_dram_parameter("out", [R, C], FP32, isOutput=True)
    att_ext = nc.declare_dram_parameter("att", [R, S], FP32, isOutput=True)

    with tile.TileContext(nc) as tc, ExitStack() as top:
        const_pool = top.enter_context(tc.tile_pool(name="const", bufs=1))
        ident = const_pool.tile([128, 128], BF16)
        make_identity(nc, ident[:])
        ident_g[0] = ident

        qk_pool = top.enter_context(tc.tile_pool(name="qk", bufs=1))
        memhi_pool = top.enter_context(tc.tile_pool(name="memhi", bufs=1))
        qT_hi = [[qk_pool.tile([128, R // 2], BF16, tag=f"qTh{h}_{m}",
                               name=f"qTh{h}_{m}") for m in range(QT)]
                 for h in range(2)]
        qT_lo = [[qk_pool.tile([128, R // 2], BF16, tag=f"qTl{h}_{m}",
                               name=f"qTl{h}_{m}") for m in range(QT)]
                 for h in range(2)]
        kT_hi = [qk_pool.tile([128, S], BF16, tag=f"kTh{m}", name=f"kTh{m}")
                 for m in range(QT)]
        kT_lo = [qk_pool.tile([128, S], BF16, tag=f"kTl{m}", name=f"kTl{m}")
                 for m in range(QT)]
        mem_hi = [memhi_pool.tile([128, C], BF16, tag=f"mh{s}", name=f"mh{s}")
                  for s in range(ST)]

        with ExitStack() as wq_stack:
            wq_pool = wq_stack.enter_context(tc.tile_pool(name="wqT", bufs=1))
            wqT_hi = [wq_pool.tile([128, Q], BF16, tag=f"h{c}", name=f"h{c}")
                      for c in range(CT)]
            wqT_lo = [wq_pool.tile([128, Q], BF16, tag=f"l{c}", name=f"l{c}")
                      for c in range(CT)]

            # ---- KEY^T first so the AllGather is in flight while the
            # x-side work keeps the engines busy ----
            with ExitStack() as wk_stack:
                wk_pool = wk_stack.enter_context(tc.tile_pool(name="wkT", bufs=1))
                wkT_hi = [wk_pool.tile([128, Q], BF16, tag=f"h{c}", name=f"h{c}")
                          for c in range(CT)]
                wkT_lo = [wk_pool.tile([128, Q], BF16, tag=f"l{c}", name=f"l{c}")
                          for c in range(CT)]
                with ExitStack() as st:
                    _load_split_transpose(nc, tc, st, wk_ext, 0, QT,
                                          wkT_hi, wkT_lo, 0, "wk")

                if SHARD_KT:
                    # compute key^T only for the S_OWN slots this core owns,
                    # AllGather the rest while X/QUERY run.
                    ccpool = top.enter_context(
                        tc.tile_pool(name="ccdram", bufs=1, space="DRAM"))
                    kin = ccpool.tile([2 * Q, S_OWN], BF16, name="kin")
                    kout = ccpool.tile([N_CORES, 2 * Q, S_OWN], BF16,
                                       name="kout", addr_space="Shared")
                    pid = nc.sync.partition_id()
                    with ExitStack() as st:
                        spool = st.enter_context(
                            tc.tile_pool(name="kslc", bufs=1))
                        pspool = st.enter_context(
                            tc.tile_pool(name="kslc_ps", bufs=2, space="PSUM"))
                        mTs_hi = [spool.tile([128, S_OWN], BF16, tag=f"th{c}",
                                             name=f"th{c}") for c in range(CT)]
                        mTs_lo = [spool.tile([128, S_OWN], BF16, tag=f"tl{c}",
                                             name=f"tl{c}") for c in range(CT)]
                        ohis, olos = [], []
                        for k2 in range(S_OWN // 128):
                            t = spool.tile([128, C], FP32, tag=f"od{k2}",
                                           name=f"od{k2}")
                            nc.sync.dma_start(
                                t[:],
                                mem_ext[bass.ds(pid * S_OWN + k2 * 128, 128), :])
                            ohi = spool.tile([128, C], BF16, tag=f"oh{k2}",
                                             name=f"oh{k2}")
                            olo = spool.tile([128, C], BF16, tag=f"ol{k2}",
                                             name=f"ol{k2}")
                            nc.scalar.copy(ohi[:], t[:])
                            nc.vector.tensor_tensor(olo[:], t[:], ohi[:],
                                                    OP.subtract)
                            ohis.append(ohi)
                            olos.append(olo)
                        for ci in range(CT):
                            for tiles, T in ((ohis, mTs_hi), (olos, mTs_lo)):
                                ps = pspool.tile([128, S_OWN], BF16, tag="ps",
                                                 name="ps")
                                for i in range(S_OWN // 128):
                                    nc.tensor.transpose(
                                        ps[:, i * 128:(i + 1) * 128],
                                        tiles[i][:, ci * 128:(ci + 1) * 128],
                                        ident[:])
                                nc.vector.tensor_copy(T[ci][:], ps[:])
                        # key^T slice + ship to bounce buffer
                        kps = st.enter_context(
                            tc.tile_pool(name="kps", bufs=2, space="PSUM"))
                        for mi in range(QT):
                            ps = kps.tile([128, S_OWN], FP32, tag="ps", name="ps")
                            n_mm = CT * 3
                            k = 0
                            for ci in range(CT):
                                for (wa, xa) in ((wkT_hi, mTs_hi),
                                                 (wkT_hi, mTs_lo),
                                                 (wkT_lo, mTs_hi)):
                                    nc.tensor.matmul(
                                        ps[:],
                                        wa[ci][:, mi * 128:(mi + 1) * 128],
                                        xa[ci][:],
                                        start=(k == 0), stop=(k == n_mm - 1))
                                    k += 1
                            sh = spool.tile([128, S_OWN], BF16, tag=f"sh{mi}",
                                            name=f"sh{mi}")
                            sl = spool.tile([128, S_OWN], BF16, tag=f"sl{mi}",
                                            name=f"sl{mi}")
                            nc.scalar.copy(sh[:], ps[:])
                            nc.vector.tensor_tensor(sl[:], ps[:], sh[:],
                                                    OP.subtract)
                            nc.sync.dma_start(
                                kin[mi * 128:(mi + 1) * 128, :], sh[:])
                            nc.sync.dma_start(
                                kin[(QT + mi) * 128:(QT + mi + 1) * 128, :], sl[:])
                        nc.gpsimd.collective_compute(
                            "AllGather", OP.bypass,
                            replica_groups=[list(range(N_CORES))],
                            ins=[kin[:]], outs=[kout[:]])
                else:
                    with ExitStack() as mst:
                        mT_pool = mst.enter_context(
                            tc.tile_pool(name="mT", bufs=1))
                        mT_hi = [mT_pool.tile([128, S // 2], BF16, tag=f"h{c}",
                                              name=f"h{c}") for c in range(CT)]
                        mT_lo = [mT_pool.tile([128, S // 2], BF16, tag=f"l{c}",
                                              name=f"l{c}") for c in range(CT)]
                        for half in range(2):
                            with ExitStack() as st:
                                _load_split_transpose(
                                    nc, tc, st, mem_ext, half * (ST // 2),
                                    ST // 2, mT_hi, mT_lo, 0, f"m{half}")
                            with ExitStack() as st:
                                _project(nc, tc, st, wkT_hi, wkT_lo,
                                         mT_hi, mT_lo, kT_hi, kT_lo,
                                         S // 2, half * (S // 2), f"k{half}")

            with ExitStack() as st:
                _load_split_transpose(nc, tc, st, wq_ext, 0, QT,
                                      wqT_hi, wqT_lo, 0, "wq")

            # ---- X + QUERY h0, then phase B rows 0..7 while the other
            # half of x is still being projected ----
            with ExitStack() as xst:
                xT_pool = xst.enter_context(tc.tile_pool(name="xT", bufs=1))
                xT_hi = [xT_pool.tile([128, R // 2], BF16, tag=f"h{c}",
                                      name=f"h{c}") for c in range(CT)]
                xT_lo = [xT_pool.tile([128, R // 2], BF16, tag=f"l{c}",
                                      name=f"l{c}") for c in range(CT)]

                def do_x_half(half):
                    with ExitStack() as st:
                        _load_split_transpose(
                            nc, tc, st, x_ext, half * (RT // 2), RT // 2,
                            xT_hi, xT_lo, 0, f"x{half}")
                    with ExitStack() as st:
                        _project(nc, tc, st, wqT_hi, wqT_lo, xT_hi, xT_lo,
                                 qT_hi[half], qT_lo[half], R // 2, 0,
                                 f"q{half}")

                do_x_half(0)

                # ---- MEM native hi (all slots; rhs of the out matmul) ----
                with tc.tile_pool(name="m_ld", bufs=3) as mload:
                    for si in range(ST):
                        t = mload.tile([128, C], FP32, tag="ld", name="ld")
                        nc.sync.dma_start(t[:],
                                          mem_ext[si * 128:(si + 1) * 128, :])
                        nc.scalar.copy(mem_hi[si][:], t[:])

                if SHARD_KT:
                    # pull the gathered key^T out of the bounce buffer
                    for split, dsts in ((0, kT_hi), (1, kT_lo)):
                        for mi in range(QT):
                            row = (split * QT + mi) * 128
                            srcap = kout[:, row:row + 128, :].rearrange(
                                "g q s -> q g s")
                            nc.sync.dma_start(
                                dsts[mi][:].rearrange("q (g s) -> q g s",
                                                      g=N_CORES), srcap)

                def b_block(ri_list):
                    with tc.tile_pool(name="batt", bufs=2) as bpool, \
                         tc.tile_pool(name="ps_att", bufs=1,
                                      space="PSUM") as ps_att_pool, \
                         tc.tile_pool(name="ps_ut", bufs=1,
                                      space="PSUM") as ps_ut_pool, \
                         tc.tile_pool(name="ps_out", bufs=1,
                                      space="PSUM") as ps_out_pool:
                        pend = None

                        def emit_post(st):
                            u, recip, ri = st["u"], st["recip"], st["ri"]
                            ps_ut = ps_ut_pool.tile([128, S], BF16, tag="ut",
                                                    name="ut")
                            for sj in range(ST):
                                nc.tensor.transpose(
                                    ps_ut[:, sj * 128:(sj + 1) * 128],
                                    u[:, sj * 128:(sj + 1) * 128], ident[:])
                            uT = bpool.tile([128, S], BF16, tag="uT", name="uT")
                            nc.vector.tensor_copy(uT[:], ps_ut[:])
                            ps_o = ps_out_pool.tile([128, C], FP32, tag="o",
                                                    name="o")
                            for nj in range(C // 512):
                                for sj in range(ST):
                                    nc.tensor.matmul(
                                        ps_o[:, nj * 512:(nj + 1) * 512],
                                        uT[:, sj * 128:(sj + 1) * 128],
                                        mem_hi[sj][:, nj * 512:(nj + 1) * 512],
                                        start=(sj == 0), stop=(sj == ST - 1))
                            out_sb = bpool.tile([128, C], FP32, tag="outsb",
                                                name="outsb")
                            nc.scalar.mul(out_sb[:], ps_o[:], recip[:])
                            nc.sync.dma_start(
                                out_ext[ri * 128:(ri + 1) * 128, :], out_sb[:])

                        for ri in ri_list:
                            ps_att = ps_att_pool.tile([128, S], FP32, tag="att",
                                                      name="att")
                            z = bpool.tile([128, S], FP32, tag="z", name="z")
                            rh, rc = ri // (RT // 2), ri % (RT // 2)
                            for sc in range(S // 512):
                                k = 0
                                n_mm = QT * 3
                                for mi in range(QT):
                                    for (qa, ka) in ((qT_hi[rh], kT_hi),
                                                     (qT_hi[rh], kT_lo),
                                                     (qT_lo[rh], kT_hi)):
                                        nc.tensor.matmul(
                                            ps_att[:, sc * 512:(sc + 1) * 512],
                                            qa[mi][:, rc * 128:(rc + 1) * 128],
                                            ka[mi][:, sc * 512:(sc + 1) * 512],
                                            start=(k == 0), stop=(k == n_mm - 1))
                                        k += 1
                                nc.scalar.activation(
                                    z[:, sc * 512:(sc + 1) * 512],
                                    ps_att[:, sc * 512:(sc + 1) * 512],
                                    AF.Exp, bias=0.0, scale=float(SCALE))

                            if pend is not None:
                                emit_post(pend)

                            t8 = bpool.tile([128, 8], FP32, tag="t8", name="t8")
                            nc.vector.max(t8[:], z[:])
                            u = bpool.tile([128, S], BF16, tag="u", name="u")
                            ssum = bpool.tile([128, 1], FP32, tag="ssum",
                                              name="ssum")
                            nc.vector.scalar_tensor_tensor(
                                out=u[:], in0=z[:], scalar=t8[:, 4:5], in1=z[:],
                                op0=OP.is_gt, op1=OP.mult, accum_out=ssum[:])
                            rin = bpool.tile([128, 1], FP32, tag="rin",
                                             name="rin")
                            nc.vector.tensor_scalar_add(rin[:], ssum[:],
                                                        float(EPS))
                            recip = bpool.tile([128, 1], FP32, tag="recip",
                                               name="recip")
                            nc.vector.reciprocal(recip[:], rin[:])
                            att_sb = bpool.tile([128, S], FP32, tag="attsb",
                                                name="attsb")
                            nc.scalar.mul(att_sb[:], u[:], recip[:])
                            nc.sync.dma_start(
                                att_ext[ri * 128:(ri + 1) * 128, :], att_sb[:])
                            pend = {"u": u, "recip": recip, "ri": ri}
                        emit_post(pend)

                b_block(range(0, RT // 2))
                do_x_half(1)
                b_block(range(RT // 2, RT))

    nc.finalize()
    return nc


_NC_CACHE = None


def kernel(x, W_q, W_k, memMatrix):
    global _NC_CACHE
    if _NC_CACHE is None:
        _NC_CACHE = build()
    nc = _NC_CACHE
    x = np.ascontiguousarray(np.asarray(x, dtype=np.float32))
    W_q = np.ascontiguousarray(np.asarray(W_q, dtype=np.float32))
    W_k = np.ascontiguousarray(np.asarray(W_k, dtype=np.float32))
    memMatrix = np.ascontiguousarray(np.asarray(memMatrix, dtype=np.float32))
    in_maps = [
        {"x": x[i * R:(i + 1) * R], "W_q": W_q, "W_k": W_k, "memMatrix": memMatrix}
        for i in range(N_CORES)
    ]
    res = run_bass_kernel_spmd(nc, in_maps, core_ids=list(range(N_CORES)))
    out = np.concatenate([res.results[i]["out"] for i in range(N_CORES)], axis=0)
    att = np.concatenate([res.results[i]["att"] for i in range(N_CORES)], axis=0)
    return out, att


# revision 23
# speedup vs baseline: 1.0615x; 1.0035x over previous
"""AttentiveMemory sparse-attention kernel for 8 TRN2 NeuronCores.

Data-parallel over rows of x (2048 rows/core); memMatrix and the projection
weights are replicated.  Per core:

    query^T = Wq^T.T @ x^T          (split-bf16 x3 matmuls, fp32 accumulate)
    key^T   = Wk^T.T @ mem^T        (split-bf16 x3; sharded over cores by
                                     memory slot + AllGather)
    logits  = query^T.T @ key^T     (split-bf16 x3) -> PSUM fp32
    z       = exp(logits/32)        (ACT, straight from PSUM; logits are tiny
                                     so no max-subtraction is needed)
    t5      = 5th largest z         (vector.max top-8, one instruction)
    u       = z * (z > t5)          (fused scalar_tensor_tensor, sum as accum)
    att     = u / (sum(u) + eps)    -> DRAM
    out     = (u @ mem) / (sum(u) + eps)  (bf16 matmul on u^T) -> DRAM

The split-bf16 trick (a@b ~ ah@bh + ah@bl + al@bh with a = ah + al) gives
fp32-grade logits at 3 bf16-rate matmuls; plain bf16/fp16/tf32 logits flip
the top-5 selection on too many rows to pass the rel-err gate.

key^T depends only on replicated inputs, so each core computes the 256
slots it owns (by partition id) and an AllGather through DRAM bounce
buffers distributes the full key^T while the x-side transposes and the
query projection keep the engines busy.
"""
from contextlib import ExitStack

import numpy as np

import concourse.bass as bass
import concourse.mybir as mybir
import concourse.tile as tile
from concourse import bacc
from concourse.masks import make_identity
from concourse.bass_utils import run_bass_kernel_spmd

N_CORES = 8
N_FULL = 16384
R = N_FULL // N_CORES   # 2048 rows per core
C = 1024                # feature dim
Q = 512                 # qk dim
S = 2048                # memory slots
SCALE = 1.0 / 32.0      # 1/sqrt(C)
EPS = 1e-12
S_OWN = S // N_CORES    # 256 slots of key^T computed per core

FP32 = mybir.dt.float32
BF16 = mybir.dt.bfloat16
AF = mybir.ActivationFunctionType
OP = mybir.AluOpType

RT = R // 128   # 16
CT = C // 128   # 8
QT = Q // 128   # 4
ST = S // 128   # 16

SHARD_KT = True

ident_g = [None]


def _load_split_transpose(nc, tc, stack, ext, row0, n_rows_tiles, T_hi, T_lo,
                          free_base, prefix, preloaded=None, ld_bufs=5,
                          sp_bufs=2):
    """Load `n_rows_tiles` (128, C) fp32 tiles of `ext` starting at row tile
    `row0`, split into bf16 hi/lo, PE-transpose into T_hi/T_lo[ci] at free
    offset `free_base`."""
    GROUP = 4
    lpool = stack.enter_context(tc.tile_pool(name=f"{prefix}_ld", bufs=ld_bufs))
    spool = stack.enter_context(tc.tile_pool(name=f"{prefix}_sp", bufs=sp_bufs))
    pspool = stack.enter_context(
        tc.tile_pool(name=f"{prefix}_ps", bufs=2, space="PSUM"))
    n_groups = n_rows_tiles // GROUP
    for g in range(n_groups):
        his, los = [], []
        for i in range(GROUP):
            rt = row0 + g * GROUP + i
            if preloaded is not None and g * GROUP + i < len(preloaded):
                t = preloaded[g * GROUP + i]
            else:
                t = lpool.tile([128, C], FP32, tag="ld", name="ld")
                nc.sync.dma_start(t[:], ext[rt * 128:(rt + 1) * 128, :])
            hi = spool.tile([128, C], BF16, tag=f"h{i}", name=f"h{i}")
            lo = spool.tile([128, C], BF16, tag=f"l{i}", name=f"l{i}")
            nc.scalar.copy(hi[:], t[:])
            nc.vector.tensor_tensor(lo[:], t[:], hi[:], OP.subtract)
            his.append(hi)
            los.append(lo)
        fb = free_base + g * GROUP * 128
        for ci in range(len(T_hi)):
            for tiles, T in ((his, T_hi), (los, T_lo)):
                ps = pspool.tile([128, GROUP * 128], BF16, tag="ps", name="ps")
                for i in range(GROUP):
                    nc.tensor.transpose(
                        ps[:, i * 128:(i + 1) * 128],
                        tiles[i][:, ci * 128:(ci + 1) * 128], ident_g[0][:])
                nc.vector.tensor_copy(T[ci][:, fb:fb + GROUP * 128], ps[:])


def _project(nc, tc, stack, wT_hi, wT_lo, inT_hi, inT_lo, oT_hi, oT_lo,
             n_free, free_base, prefix, chunk=512):
    """o^T = W^T.T @ in^T with split-bf16 x3; writes oT_*[mi][:, free_base:...]."""
    pspool = stack.enter_context(
        tc.tile_pool(name=f"{prefix}_qps", bufs=4, space="PSUM"))
    ct = len(wT_hi)
    for mi in range(len(oT_hi)):
        for ni in range(n_free // chunk):
            ps = pspool.tile([128, chunk], FP32, tag="ps", name="ps")
            n_mm = ct * 3
            k = 0
            for ci in range(ct):
                for (wa, xa) in ((wT_hi, inT_hi), (wT_hi, inT_lo),
                                 (wT_lo, inT_hi)):
                    nc.tensor.matmul(
                        ps[:],
                        wa[ci][:, mi * 128:(mi + 1) * 128],
                        xa[ci][:, ni * chunk:(ni + 1) * chunk],
                        start=(k == 0), stop=(k == n_mm - 1))
                    k += 1
            dst = slice(free_base + ni * chunk, free_base + (ni + 1) * chunk)
            nc.scalar.copy(oT_hi[mi][:, dst], ps[:])
            nc.vector.tensor_tensor(
                oT_lo[mi][:, dst], ps[:], oT_hi[mi][:, dst], OP.subtract)


def build():
    nc = bacc.Bacc()
    x_ext = nc.declare_dram_parameter("x", [R, C], FP32, isOutput=False)
    wq_ext = nc.declare_dram_parameter("W_q", [Q, C], FP32, isOutput=False)
    wk_ext = nc.declare_dram_parameter("W_k", [Q, C], FP32, isOutput=False)
    mem_ext = nc.declare_dram_parameter("memMatrix", [S, C], FP32, isOutput=False)
    out_ext = nc.declare_dram_parameter("out", [R, C], FP32, isOutput=True)
    att_ext = nc.declare_dram_parameter("att", [R, S], FP32, isOutput=True)

    with tile.TileContext(nc) as tc, ExitStack() as top:
        const_pool = top.enter_context(tc.tile_pool(name="const", bufs=1))
        ident = const_pool.tile([128, 128], BF16)
        make_identity(nc, ident[:])
        ident_g[0] = ident

        qk_pool = top.enter_context(tc.tile_pool(name="qk", bufs=1))
        memhi_pool = top.enter_context(tc.tile_pool(name="memhi", bufs=1))
        qT_hi = [[qk_pool.tile([128, R // 2], BF16, tag=f"qTh{h}_{m}",
                               name=f"qTh{h}_{m}") for m in range(QT)]
                 for h in range(2)]
        qT_lo = [[qk_pool.tile([128, R // 2], BF16, tag=f"qTl{h}_{m}",
                               name=f"qTl{h}_{m}") for m in range(QT)]
                 for h in range(2)]
        kT_hi = [qk_pool.tile([128, S], BF16, tag=f"kTh{m}", name=f"kTh{m}")
                 for m in range(QT)]
        kT_lo = [qk_pool.tile([128, S], BF16, tag=f"kTl{m}", name=f"kTl{m}")
                 for m in range(QT)]
        mem_hi = [memhi_pool.tile([128, C], BF16, tag=f"mh{s}", name=f"mh{s}")
                  for s in range(ST)]

        with ExitStack() as wq_stack:
            wq_pool = wq_stack.enter_context(tc.tile_pool(name="wqT", bufs=1))
            wqT_hi = [wq_pool.tile([128, Q], BF16, tag=f"h{c}", name=f"h{c}")
                      for c in range(CT)]
            wqT_lo = [wq_pool.tile([128, Q], BF16, tag=f"l{c}", name=f"l{c}")
                      for c in range(CT)]

            # ---- KEY^T first so the AllGather is in flight while the
            # x-side work keeps the engines busy ----
            with ExitStack() as wk_stack:
                wk_pool = wk_stack.enter_context(tc.tile_pool(name="wkT", bufs=1))
                wkT_hi = [wk_pool.tile([128, Q], BF16, tag=f"h{c}", name=f"h{c}")
                          for c in range(CT)]
                wkT_lo = [wk_pool.tile([128, Q], BF16, tag=f"l{c}", name=f"l{c}")
                          for c in range(CT)]
                with ExitStack() as st:
                    _load_split_transpose(nc, tc, st, wk_ext, 0, QT,
                                          wkT_hi, wkT_lo, 0, "wk")

                if SHARD_KT:
                    # compute key^T only for the S_OWN slots this core owns,
                    # AllGather the rest while X/QUERY run.
                    ccpool = top.enter_context(
                        tc.tile_pool(name="ccdram", bufs=1, space="DRAM"))
                    kin = ccpool.tile([2 * Q, S_OWN], BF16, name="kin")
                    kout = ccpool.tile([N_CORES, 2 * Q, S_OWN], BF16,
                                       name="kout", addr_space="Shared")
                    pid = nc.sync.partition_id()
                    with ExitStack() as st:
                        spool = st.enter_context(
                            tc.tile_pool(name="kslc", bufs=1))
                        pspool = st.enter_context(
                            tc.tile_pool(name="kslc_ps", bufs=2, space="PSUM"))
                        mTs_hi = [spool.tile([128, S_OWN], BF16, tag=f"th{c}",
                                             name=f"th{c}") for c in range(CT)]
                        mTs_lo = [spool.tile([128, S_OWN], BF16, tag=f"tl{c}",
                                             name=f"tl{c}") for c in range(CT)]
                        ohis, olos = [], []
                        for k2 in range(S_OWN // 128):
                            t = spool.tile([128, C], FP32, tag=f"od{k2}",
                                           name=f"od{k2}")
                            nc.sync.dma_start(
                                t[:],
                                mem_ext[bass.ds(pid * S_OWN + k2 * 128, 128), :])
                            ohi = spool.tile([128, C], BF16, tag=f"oh{k2}",
                                             name=f"oh{k2}")
                            olo = spool.tile([128, C], BF16, tag=f"ol{k2}",
                                             name=f"ol{k2}")
                            nc.scalar.copy(ohi[:], t[:])
                            nc.vector.tensor_tensor(olo[:], t[:], ohi[:],
                                                    OP.subtract)
                            ohis.append(ohi)
                            olos.append(olo)
                        for ci in range(CT):
                            for tiles, T in ((ohis, mTs_hi), (olos, mTs_lo)):
                                ps = pspool.tile([128, S_OWN], BF16, tag="ps",
                                                 name="ps")
                                for i in range(S_OWN // 128):
                                    nc.tensor.transpose(
                                        ps[:, i * 128:(i + 1) * 128],
                                        tiles[i][:, ci * 128:(ci + 1) * 128],
                                        ident[:])
                                nc.vector.tensor_copy(T[ci][:], ps[:])
                        # key^T slice + ship to bounce buffer
                        kps = st.enter_context(
                            tc.tile_pool(name="kps", bufs=2, space="PSUM"))
                        for mi in range(QT):
                            ps = kps.tile([128, S_OWN], FP32, tag="ps", name="ps")
                            n_mm = CT * 3
                            k = 0
                            for ci in range(CT):
                                for (wa, xa) in ((wkT_hi, mTs_hi),
                                                 (wkT_hi, mTs_lo),
                                                 (wkT_lo, mTs_hi)):
                                    nc.tensor.matmul(
                                        ps[:],
                                        wa[ci][:, mi * 128:(mi + 1) * 128],
                                        xa[ci][:],
                                        start=(k == 0), stop=(k == n_mm - 1))
                                    k += 1
                            sh = spool.tile([128, S_OWN], BF16, tag=f"sh{mi}",
                                            name=f"sh{mi}")
                            sl = spool.tile([128, S_OWN], BF16, tag=f"sl{mi}",
                                            name=f"sl{mi}")
                            nc.scalar.copy(sh[:], ps[:])
                            nc.vector.tensor_tensor(sl[:], ps[:], sh[:],
                                                    OP.subtract)
                            nc.sync.dma_start(
                                kin[mi * 128:(mi + 1) * 128, :], sh[:])
                            nc.sync.dma_start(
                                kin[(QT + mi) * 128:(QT + mi + 1) * 128, :], sl[:])
                        nc.gpsimd.collective_compute(
                            "AllGather", OP.bypass,
                            replica_groups=[list(range(N_CORES))],
                            ins=[kin[:]], outs=[kout[:]])
                else:
                    with ExitStack() as mst:
                        mT_pool = mst.enter_context(
                            tc.tile_pool(name="mT", bufs=1))
                        mT_hi = [mT_pool.tile([128, S // 2], BF16, tag=f"h{c}",
                                              name=f"h{c}") for c in range(CT)]
                        mT_lo = [mT_pool.tile([128, S // 2], BF16, tag=f"l{c}",
                                              name=f"l{c}") for c in range(CT)]
                        for half in range(2):
                            with ExitStack() as st:
                                _load_split_transpose(
                                    nc, tc, st, mem_ext, half * (ST // 2),
                                    ST // 2, mT_hi, mT_lo, 0, f"m{half}")
                            with ExitStack() as st:
                                _project(nc, tc, st, wkT_hi, wkT_lo,
                                         mT_hi, mT_lo, kT_hi, kT_lo,
                                         S // 2, half * (S // 2), f"k{half}")

            with ExitStack() as st:
                _load_split_transpose(nc, tc, st, wq_ext, 0, QT,
                                      wqT_hi, wqT_lo, 0, "wq")

            # ---- X + QUERY h0, then phase B rows 0..7 while the other
            # half of x is still being projected ----
            with ExitStack() as xst:
                xT_pool = xst.enter_context(tc.tile_pool(name="xT", bufs=1))
                xT_hi = [xT_pool.tile([128, R // 2], BF16, tag=f"h{c}",
                                      name=f"h{c}") for c in range(CT)]
                xT_lo = [xT_pool.tile([128, R // 2], BF16, tag=f"l{c}",
                                      name=f"l{c}") for c in range(CT)]

                def do_x_half(half):
                    with ExitStack() as st:
                        _load_split_transpose(
                            nc, tc, st, x_ext, half * (RT // 2), RT // 2,
                            xT_hi, xT_lo, 0, f"x{half}",
                            preloaded=x1_tiles if half == 1 else None)
                    with ExitStack() as st:
                        _project(nc, tc, st, wqT_hi, wqT_lo, xT_hi, xT_lo,
                                 qT_hi[half], qT_lo[half], R // 2, 0,
                                 f"q{half}")

                do_x_half(0)

                # ---- MEM native hi (all slots; rhs of the out matmul) ----
                with tc.tile_pool(name="m_ld", bufs=3) as mload:
                    for si in range(ST):
                        t = mload.tile([128, C], FP32, tag="ld", name="ld")
                        nc.sync.dma_start(t[:],
                                          mem_ext[si * 128:(si + 1) * 128, :])
                        nc.scalar.copy(mem_hi[si][:], t[:])

                if SHARD_KT:
                    # pull the gathered key^T out of the bounce buffer
                    for split, dsts in ((0, kT_hi), (1, kT_lo)):
                        for mi in range(QT):
                            row = (split * QT + mi) * 128
                            srcap = kout[:, row:row + 128, :].rearrange(
                                "g q s -> q g s")
                            nc.sync.dma_start(
                                dsts[mi][:].rearrange("q (g s) -> q g s",
                                                      g=N_CORES), srcap)

                def b_block(ri_list):
                    with tc.tile_pool(name="batt", bufs=2) as bpool, \
                         tc.tile_pool(name="batt1", bufs=1) as bpool1, \
                         tc.tile_pool(name="ps_att", bufs=1,
                                      space="PSUM") as ps_att_pool, \
                         tc.tile_pool(name="ps_ut", bufs=1,
                                      space="PSUM") as ps_ut_pool, \
                         tc.tile_pool(name="ps_out", bufs=1,
                                      space="PSUM") as ps_out_pool:
                        pend = None

                        def emit_post(st):
                            u, recip, ri = st["u"], st["recip"], st["ri"]
                            ps_ut = ps_ut_pool.tile([128, S], BF16, tag="ut",
                                                    name="ut")
                            for sj in range(ST):
                                nc.tensor.transpose(
                                    ps_ut[:, sj * 128:(sj + 1) * 128],
                                    u[:, sj * 128:(sj + 1) * 128], ident[:])
                            uT = bpool1.tile([128, S], BF16, tag="uT", name="uT")
                            nc.vector.tensor_copy(uT[:], ps_ut[:])
                            ps_o = ps_out_pool.tile([128, C], FP32, tag="o",
                                                    name="o")
                            for nj in range(C // 512):
                                for sj in range(ST):
                                    nc.tensor.matmul(
                                        ps_o[:, nj * 512:(nj + 1) * 512],
                                        uT[:, sj * 128:(sj + 1) * 128],
                                        mem_hi[sj][:, nj * 512:(nj + 1) * 512],
                                        start=(sj == 0), stop=(sj == ST - 1))
                            out_sb = bpool1.tile([128, C], FP32, tag="outsb",
                                                 name="outsb")
                            nc.scalar.mul(out_sb[:], ps_o[:], recip[:])
                            nc.sync.dma_start(
                                out_ext[ri * 128:(ri + 1) * 128, :], out_sb[:])

                        for ri in ri_list:
                            ps_att = ps_att_pool.tile([128, S], FP32, tag="att",
                                                      name="att")
                            z = bpool.tile([128, S], FP32, tag="z", name="z")
                            rh, rc = ri // (RT // 2), ri % (RT // 2)
                            for sc in range(S // 512):
                                k = 0
                                n_mm = QT * 3
                                for mi in range(QT):
                                    for (qa, ka) in ((qT_hi[rh], kT_hi),
                                                     (qT_hi[rh], kT_lo),
                                                     (qT_lo[rh], kT_hi)):
                                        nc.tensor.matmul(
                                            ps_att[:, sc * 512:(sc + 1) * 512],
                                            qa[mi][:, rc * 128:(rc + 1) * 128],
                                            ka[mi][:, sc * 512:(sc + 1) * 512],
                                            start=(k == 0), stop=(k == n_mm - 1))
                                        k += 1
                                nc.scalar.activation(
                                    z[:, sc * 512:(sc + 1) * 512],
                                    ps_att[:, sc * 512:(sc + 1) * 512],
                                    AF.Exp, bias=0.0, scale=float(SCALE))

                            if pend is not None:
                                emit_post(pend)

                            t8 = bpool.tile([128, 8], FP32, tag="t8", name="t8")
                            nc.vector.max(t8[:], z[:])
                            u = bpool.tile([128, S], BF16, tag="u", name="u")
                            ssum = bpool.tile([128, 1], FP32, tag="ssum",
                                              name="ssum")
                            nc.vector.scalar_tensor_tensor(
                                out=u[:], in0=z[:], scalar=t8[:, 4:5], in1=z[:],
                                op0=OP.is_gt, op1=OP.mult, accum_out=ssum[:])
                            rin = bpool.tile([128, 1], FP32, tag="rin",
                                             name="rin")
                            nc.vector.tensor_scalar_add(rin[:], ssum[:],
                                                        float(EPS))
                            recip = bpool.tile([128, 1], FP32, tag="recip",
                                               name="recip")
                            nc.vector.reciprocal(recip[:], rin[:])
                            att_sb = bpool1.tile([128, S], FP32, tag="attsb",
                                                 name="attsb")
                            nc.scalar.mul(att_sb[:], u[:], recip[:])
                            nc.sync.dma_start(
                                att_ext[ri * 128:(ri + 1) * 128, :], att_sb[:])
                            pend = {"u": u, "recip": recip, "ri": ri}
                        emit_post(pend)

                x1_stack = ExitStack()
                x1_pool = x1_stack.enter_context(
                    tc.tile_pool(name="x1_ld", bufs=1, side="right"))
                x1_tiles = []
                for i in range(2):
                    rt = RT // 2 + i
                    t1 = x1_pool.tile([128, C], FP32, tag=f"x1_{i}",
                                      name=f"x1_{i}")
                    nc.sync.dma_start(t1[:], x_ext[rt * 128:(rt + 1) * 128, :])
                    x1_tiles.append(t1)
                b_block(range(0, RT // 2))
                do_x_half(1)
                x1_stack.close()
                b_block(range(RT // 2, RT))

    nc.finalize()
    return nc


_NC_CACHE = None


def kernel(x, W_q, W_k, memMatrix):
    global _NC_CACHE
    if _NC_CACHE is None:
        _NC_CACHE = build()
    nc = _NC_CACHE
    x = np.ascontiguousarray(np.asarray(x, dtype=np.float32))
    W_q = np.ascontiguousarray(np.asarray(W_q, dtype=np.float32))
    W_k = np.ascontiguousarray(np.asarray(W_k, dtype=np.float32))
    memMatrix = np.ascontiguousarray(np.asarray(memMatrix, dtype=np.float32))
    in_maps = [
        {"x": x[i * R:(i + 1) * R], "W_q": W_q, "W_k": W_k, "memMatrix": memMatrix}
        for i in range(N_CORES)
    ]
    res = run_bass_kernel_spmd(nc, in_maps, core_ids=list(range(N_CORES)))
    out = np.concatenate([res.results[i]["out"] for i in range(N_CORES)], axis=0)
    att = np.concatenate([res.results[i]["att"] for i in range(N_CORES)], axis=0)
    return out, att
